# revision 8
# baseline (speedup 1.0000x reference)
"""Trainium2 Bass kernel for nn_MoLE (moe_routing), v2.

Reference computation (TOPK=1, softmax over 1 element == 1.0):
  out[:, 0:32]  = sigmoid(conv2(relu(conv1(rgb_local, Wsel_rgb)))) +
                  sigmoid(conv2(relu(conv1(ir_local,  Wsel_ir))))
  out[:, 32:96] = concat(rgb_dense, ir_dense)   (identity -> host assembles)

Device computes ONLY the local-feature channels; the dense passthrough is
pure identity and is assembled on the host, halving device HBM traffic.

Sharding: rows (H) split across 8 cores, 60 output rows each. Host gating
(argmax of 2 scores per sample/branch); selected expert conv params packed
per-core. Inputs pre-cast to bf16 and pre-padded (reflect) on host.

Per-core geometry per wave (p = sample pair, q = quarter of 15 out rows):
  4 units g = 2 samples x 2 branches. Unit g owns partition band 32g.
  conv1: x [32, 19 rows x 642] bf16 -> 22 free-chunks x 9 taps, PE tile
  (32g, 32g), PSUM [16, <=512]; relu+bias -> B0_g [16, 17 rows x 642] bf16.
  conv2: taps read B0_g at free offsets (dy*WP+dx) -> 20 chunks x 9 taps,
  PE tile (32g, 32sc) with the branch pair sharing col band sc (distinct
  PSUM banks); sigmoid+bias -> y; pair add -> bf16 out strip.
"""
import os
import sys

sys.path.insert(0, "/opt/trn_rl_repo")
sys.path.insert(0, os.path.dirname(os.path.abspath(__file__)))

import numpy as np
import ml_dtypes

import concourse.bass as bass
import concourse.mybir as mybir
from concourse import tile
from concourse.tile import add_dep_helper
from concourse.bass_utils import run_bass_kernel_spmd


def split_waits(nc, max_waits=1):
    for bb in nc.main_func.blocks:
        new_insts = []
        for ins in bb.instructions:
            si = ins.sync_info
            if si is not None and si.on_wait and len(si.on_wait) > max_waits:
                waits = list(si.on_wait)
                extra, keep = waits[:-max_waits], waits[-max_waits:]
                for i in range(0, len(extra), max_waits):
                    nop = mybir.InstNoOp(
                        name=nc.get_next_instruction_name(),
                        engine=ins.engine,
                        ins=[], outs=[],
                        sync_info=mybir.SyncInfo(
                            on_wait=extra[i:i + max_waits], on_update=[]),
                        bass_nofuse=True,
                    )
                    new_insts.append(nop)
                si.on_wait = keep
            new_insts.append(ins)
        bb.instructions[:] = new_insts
    return nc


def merge_pe_incs(nc):
    """Drop per-matmul semaphore increments on non-stop matmuls; rewrite all
    waits on those sems to the thinner counting."""
    all_insts = []
    for bb in nc.main_func.blocks:
        all_insts.extend(bb.instructions)
    mm_sems = set()
    for ins in all_insts:
        if type(ins).__name__ == "InstMatmult" and ins.sync_info:
            for u in (ins.sync_info.on_update or []):
                if u.update_mode == "sem-inc" and u.update_reg is None:
                    mm_sems.add(u.id)
    if not mm_sems:
        return nc
    o2n = {sid: {0: 0} for sid in mm_sems}
    oc = {sid: 0 for sid in mm_sems}
    ncnt = {sid: 0 for sid in mm_sems}
    for ins in all_insts:
        si = ins.sync_info
        if si is None or not si.on_update:
            continue
        drop = (type(ins).__name__ == "InstMatmult"
                and not ins.stop_tensor_calc)
        keep = []
        for u in si.on_update:
            if (u.id in mm_sems and u.update_mode == "sem-inc"
                    and u.update_reg is None):
                oc[u.id] += u.update_value
                if drop:
                    o2n[u.id][oc[u.id]] = ncnt[u.id] + 1
                else:
                    ncnt[u.id] += u.update_value
                    o2n[u.id][oc[u.id]] = ncnt[u.id]
                    keep.append(u)
            else:
                keep.append(u)
        si.on_update = keep
    for ins in all_insts:
        si = ins.sync_info
        if si is None or not si.on_wait:
            continue
        for wt in si.on_wait:
            if wt.id in mm_sems and wt.wait_mode == "sem-ge-imm" \
                    and wt.wait_value is not None:
                wt.wait_value = min(
                    o2n[wt.id].get(wt.wait_value, ncnt[wt.id]),
                    ncnt[wt.id])
    return nc


BF16 = mybir.dt.bfloat16
F32 = mybir.dt.float32

B, C, H, W, E = 4, 32, 480, 640, 4
CH = 16              # hidden channels (C//2)
N_CORES = 8
ROWS = H // N_CORES  # 60 output rows per core
WP = W + 2           # padded row length 642
XROWS = ROWS + 5     # 65 padded input rows per core strip
NQ = 4               # quarters per strip
QR = 15              # output rows per quarter
C1R = QR + 2         # conv1 rows per quarter (17)
XQR = QR + 4         # x rows per quarter (19)
C1LEN = C1R * WP     # 10914 conv1 elems per quarter
C2LEN = QR * WP      # 9630 conv2 elems per quarter
XJ2 = XQR * WP + 2   # x tile free size (1-elem guards both ends)

# conv1 free chunks (PSUM bank = 512 fp32)
C1CH = [496] * 21 + [C1LEN - 21 * 496]          # 21x496 + 498
C1OFF = [496 * i for i in range(22)]
# conv2 free chunks
C2CH = [496] * 19 + [C2LEN - 19 * 496]          # 19x496 + 206
C2OFF = [496 * i for i in range(20)]

TAPS = [(dy, dx) for dy in (-1, 0, 1) for dx in (-1, 0, 1)]


def build_kernel():
    nc = bass.Bass()
    xpad = nc.declare_dram_parameter("xpad", [B, 2, C, XROWS, WP], BF16,
                                     isOutput=False)
    w1s = nc.declare_dram_parameter("w1s", [4, C, 2 * 9 * CH], BF16,
                                    isOutput=False)
    w2s = nc.declare_dram_parameter("w2s", [4, CH, 8 * 9 * C], BF16,
                                    isOutput=False)
    b1r = nc.declare_dram_parameter("b1r", [128, 8], F32, isOutput=False)
    b2r = nc.declare_dram_parameter("b2r", [128, 8], F32, isOutput=False)
    emask = nc.declare_dram_parameter("emask", [128, 2], F32, isOutput=False)
    # full padded width; host strips the 2 junk cols
    out = nc.declare_dram_parameter("out", [B, C, ROWS * WP], BF16,
                                    isOutput=True)

    from contextlib import ExitStack
    with tile.TileContext(nc) as tc, ExitStack() as es:
        wpool = es.enter_context(tc.tile_pool(name="wpool", bufs=1))
        xpool = es.enter_context(tc.tile_pool(name="xpool", bufs=2))
        rpool = es.enter_context(tc.tile_pool(name="rpool", bufs=2))
        ypool = es.enter_context(tc.tile_pool(name="ypool", bufs=2))
        p1pool = es.enter_context(tc.tile_pool(name="p1pool", bufs=2,
                                               space="PSUM"))
        p2pool = es.enter_context(tc.tile_pool(name="p2pool", bufs=2,
                                               space="PSUM"))

        # --- preload weights/biases ---
        w1t = wpool.tile([128, 2 * 9 * CH], BF16, tag="w1")
        w2t = wpool.tile([128, 8 * 9 * C], BF16, tag="w2")
        b1t = wpool.tile([128, 8], F32, tag="b1")
        b2t = wpool.tile([128, 8], F32, tag="b2")
        emt = wpool.tile([128, 2], F32, tag="em")
        zt = wpool.tile([128, 512], F32, tag="zt")
        for g in range(4):
            nc.sync.dma_start(w1t[32 * g:32 * g + 32, :], w1s[g])
            nc.sync.dma_start(w2t[32 * g:32 * g + CH, :], w2s[g])
        nc.sync.dma_start(b1t[:, :], b1r[:, :])
        nc.sync.dma_start(b2t[:, :], b2r[:, :])
        nc.sync.dma_start(emt[:, :], emask[:, :])
        nc.gpsimd.memset(zt[:, :], 0.0)

        prev_mm = None

        for w in range(8):
            p, q = divmod(w, NQ)
            units = [(2 * p, 0), (2 * p, 1), (2 * p + 1, 0), (2 * p + 1, 1)]
            uidx = [b * 2 + br for (b, br) in units]

            # --- x loads (bf16, contiguous rows 15q .. 15q+18) ---
            xt = xpool.tile([128, XJ2], BF16, tag="x")
            for g, (b, br) in enumerate(units):
                src = xpad[b, br, :, QR * q:QR * q + XQR, :]
                nc.sync.dma_start(
                    xt[32 * g:32 * g + 32, 1:1 + XQR * WP],
                    src.rearrange("c r w -> c (r w)"))

            # --- conv1: 22 chunks x 9 taps x 4 units ---
            rt = rpool.tile([128, 1 + C1LEN + 1], BF16, tag="b0")
            for cc in range(22):
                ln = C1CH[cc]
                o1 = C1OFF[cc]
                T = p1pool.tile([128, 512], F32, tag="p1", name=f"T{w}_{cc}")
                for t, (dy, dx) in enumerate(TAPS):
                    for g in range(4):
                        lhs = w1t[32 * g:32 * g + 32,
                                  ((p * 9 + t) * CH):((p * 9 + t) * CH) + CH]
                        base = 1 + o1 + (dy + 1) * WP + dx
                        mm = nc.tensor.matmul(
                            T[32 * g:32 * g + CH, 0:ln],
                            lhs,
                            xt[32 * g:32 * g + 32, base:base + ln],
                            start=(t == 0), stop=(t == 8),
                            tile_position=(32 * g, 32 * g),
                        )
                        if prev_mm is not None:
                            add_dep_helper(mm.ins, prev_mm.ins, sync=False,
                                           reason="pe-order")
                        prev_mm = mm
                # relu+bias; g=0 on Act engine, g=1..3 on DVE
                for g in range(4):
                    dst = rt[32 * g:32 * g + CH, 1 + o1:1 + o1 + ln]
                    src = T[32 * g:32 * g + CH, 0:ln]
                    if g == 0:
                        nc.scalar.activation(
                            dst, src, mybir.ActivationFunctionType.Relu,
                            bias=b1t[32 * g:32 * g + CH,
                                     uidx[g]:uidx[g] + 1])
                    else:
                        nc.vector.scalar_tensor_tensor(
                            out=dst, in0=src,
                            scalar=b1t[32 * g:32 * g + CH,
                                       uidx[g]:uidx[g] + 1],
                            in1=zt[32 * g:32 * g + CH, 0:ln],
                            op0=mybir.AluOpType.add,
                            op1=mybir.AluOpType.max)

            # --- conv1-out fixes ---
            # (a) global-image row reflection of the conv1 feature map:
            # core 0 q0: frow0 (conv1 row -1) := frow2; core 7 q3:
            # frow16 (row 60) := frow14. Data-driven per-core mask.
            if q == 0 or q == NQ - 1:
                fbad = 0 if q == 0 else 16
                fgood = 2 if q == 0 else 14
                mcol = 0 if q == 0 else 1
                for g in range(4):
                    bad = rt[32 * g:32 * g + CH,
                             1 + fbad * WP:1 + (fbad + 1) * WP]
                    good = rt[32 * g:32 * g + CH,
                              1 + fgood * WP:1 + (fgood + 1) * WP]
                    etmp = wpool.tile([128, WP], BF16, tag="etmp",
                                      name=f"etmp{w}_{g}", bufs=2)
                    et = etmp[32 * g:32 * g + CH, :]
                    nc.vector.tensor_sub(et, good, bad)
                    nc.vector.scalar_tensor_tensor(
                        out=bad, in0=et,
                        scalar=emt[32 * g:32 * g + CH, mcol:mcol + 1],
                        in1=bad,
                        op0=mybir.AluOpType.mult, op1=mybir.AluOpType.add)
            # (b) conv1-out col reflect: span col0 := col2, col641 := col639
            # (conv2's dx=+-1 taps at valid edge cols read them)
            for g in range(4):
                nc.vector.tensor_copy(
                    rt[32 * g:32 * g + CH, 1:1 + (C1R - 1) * WP + 1:WP],
                    rt[32 * g:32 * g + CH, 3:3 + (C1R - 1) * WP + 1:WP])
                nc.vector.tensor_copy(
                    rt[32 * g:32 * g + CH, WP:(C1R) * WP + 1:WP],
                    rt[32 * g:32 * g + CH, W:W + (C1R - 1) * WP + 1:WP])

            # --- conv2: 20 chunks x 9 taps x 4 units; pairs share col band
            # sc, distinct PSUM banks (tags A/B) ---
            for c2 in range(20):
                ln = C2CH[c2]
                o2 = C2OFF[c2]
                sc = c2 % 4
                UA = p2pool.tile([128, 512], F32, tag="p2a", name=f"UA{w}_{c2}")
                UB = p2pool.tile([128, 512], F32, tag="p2b", name=f"UB{w}_{c2}")
                # g=0,1 (sample 2p) both at band sc; g=2,3 at band sc2
                sc2 = (sc + 2) % 4
                bands = [sc, sc, sc2, sc2]
                tags = [UA, UB, UA, UB]
                for t, (dy, dx) in enumerate(TAPS):
                    base2 = 1 + o2 + (dy + 1) * WP + dx
                    for g in range(4):
                        bb = bands[g]
                        mm = nc.tensor.matmul(
                            tags[g][32 * bb:32 * bb + C, 0:ln],
                            w2t[32 * g:32 * g + CH,
                                (uidx[g] * 9 + t) * C:
                                (uidx[g] * 9 + t) * C + C],
                            rt[32 * g:32 * g + CH, base2:base2 + ln],
                            start=(t == 0), stop=(t == 8),
                            tile_position=(32 * g, 32 * bb),
                        )
                        if prev_mm is not None:
                            add_dep_helper(mm.ins, prev_mm.ins, sync=False,
                                           reason="pe-order")
                        prev_mm = mm
                # sigmoid (Act) then pair add (DVE), per sample pair
                yA = ypool.tile([128, 512], BF16, tag="ya", name=f"yA{w}_{c2}")
                yB = ypool.tile([128, 512], BF16, tag="yb", name=f"yB{w}_{c2}")
                ot = ypool.tile([128, 512], BF16, tag="o", name=f"o{w}_{c2}")
                for pair in range(2):
                    g0, g1 = 2 * pair, 2 * pair + 1
                    bb = bands[g0]
                    sl = slice(32 * bb, 32 * bb + C)
                    nc.scalar.activation(
                        yA[sl, 0:ln], tags[g0][sl, 0:ln],
                        mybir.ActivationFunctionType.Sigmoid,
                        bias=b2t[sl, uidx[g0]:uidx[g0] + 1])
                    nc.scalar.activation(
                        yB[sl, 0:ln], tags[g1][sl, 0:ln],
                        mybir.ActivationFunctionType.Sigmoid,
                        bias=b2t[sl, uidx[g1]:uidx[g1] + 1])
                    nc.vector.tensor_tensor(
                        out=ot[sl, 0:ln], in0=yA[sl, 0:ln], in1=yB[sl, 0:ln],
                        op=mybir.AluOpType.add)
                    # out DMA: one contiguous transfer (padded width; host
                    # strips junk cols)
                    b = units[g0][0]
                    nc.sync.dma_start(
                        out[b, :, QR * q * WP + o2:QR * q * WP + o2 + ln],
                        ot[sl, 0:ln])

    merge_pe_incs(nc)
    split_waits(nc)
    return nc


def _host_gate_and_pack(inputs):
    """Host-side gating (argmax over 2 scores per sample/branch) and packing
    of selected expert conv params into device layouts."""
    rl = inputs["rgb_local"]
    il = inputs["ir_local"]
    sc_rgb = rl.reshape(B, -1) @ inputs["gate_rgb_w"].reshape(2, -1).T \
        + inputs["gate_rgb_b"]
    sc_ir = il.reshape(B, -1) @ inputs["gate_ir_w"].reshape(2, -1).T \
        + inputs["gate_ir_b"]
    e_rgb = np.argmax(sc_rgb, axis=1)          # in {0,1}
    e_ir = np.argmax(sc_ir, axis=1) + 2        # in {2,3}

    ew1, eb1 = inputs["ew1"], inputs["eb1"]    # [E,16,32,3,3], [E,16]
    ew2, eb2 = inputs["ew2"], inputs["eb2"]    # [E,32,16,3,3], [E,32]

    eu = np.empty(8, np.int64)                 # unit u = b*2+br
    for b in range(B):
        eu[b * 2 + 0] = e_rgb[b]
        eu[b * 2 + 1] = e_ir[b]

    # w1[g, ci, p, tap, co] = ew1[eu(u(g,p)), co, ci, tap]
    w1 = np.empty((4, C, 2, 9, CH), np.float32)
    for g in range(4):
        for p in range(2):
            u = (2 * p + g // 2) * 2 + (g % 2)
            sel = ew1[eu[u]]                   # [16, 32, 3, 3]
            w1[g, :, p] = np.moveaxis(sel.reshape(CH, C, 9),
                                      [0, 1, 2], [2, 0, 1])

    w2 = np.empty((4, CH, 8, 9, C), np.float32)
    for u in range(8):
        sel = ew2[eu[u]]                       # [32, 16, 3, 3]
        m = np.moveaxis(sel.reshape(C, CH, 9), [0, 1, 2], [2, 0, 1])
        for s in range(4):
            w2[s, :, u] = m

    b1 = np.zeros((128, 8), np.float32)
    b2 = np.zeros((128, 8), np.float32)
    for u in range(8):
        for s in range(4):
            b1[32 * s:32 * s + CH, u] = eb1[eu[u]]
            b2[32 * s:32 * s + C, u] = eb2[eu[u]]

    return (w1.reshape(4, C, 2 * 9 * CH).astype(ml_dtypes.bfloat16),
            w2.reshape(4, CH, 8 * 9 * C).astype(ml_dtypes.bfloat16),
            b1, b2)


def _build_xpad(x, core):
    """[B, C, H, W] fp32 -> padded strip [B, C, XROWS, WP] for one core."""
    r0 = ROWS * core - 2
    rows = np.arange(r0, r0 + XROWS)
    rows = np.where(rows < 0, -rows, rows)
    rows = np.where(rows >= H, 2 * (H - 1) - rows, rows)
    strip = x[:, :, rows, :]                       # [B, C, XROWS, W]
    padded = np.empty((B, C, XROWS, WP), np.float32)
    padded[:, :, :, 1:W + 1] = strip
    padded[:, :, :, 0] = strip[:, :, :, 1]
    padded[:, :, :, W + 1] = strip[:, :, :, W - 2]
    return padded


_CACHE = {}


def _get_nc():
    if "nc" not in _CACHE:
        _CACHE["nc"] = build_kernel()
    return _CACHE["nc"]


def make_in_maps(inputs):
    w1, w2, b1, b2 = _host_gate_and_pack(inputs)
    xp_rgb = inputs["rgb_local"]
    xp_ir = inputs["ir_local"]
    in_maps = []
    for core in range(N_CORES):
        em = np.zeros((128, 2), np.float32)
        if core == 0:
            em[:, 0] = 1.0
        if core == N_CORES - 1:
            em[:, 1] = 1.0
        xpad = np.stack(
            [_build_xpad(xp_rgb, core), _build_xpad(xp_ir, core)],
            axis=1).astype(ml_dtypes.bfloat16)
        in_maps.append(dict(
            xpad=np.ascontiguousarray(xpad), emask=em,
            w1s=w1, w2s=w2, b1r=b1, b2r=b2,
        ))
    return in_maps


def _assemble(inputs, results):
    strips = [results[i]["out"].reshape(B, C, ROWS, WP)[:, :, :, 1:W + 1]
              for i in range(N_CORES)]
    local = np.concatenate(strips, axis=2).astype(np.float32)
    return np.concatenate(
        [local, np.asarray(inputs["rgb_dense"], np.float32),
         np.asarray(inputs["ir_dense"], np.float32)], axis=1)


def kernel(**inputs):
    inputs = {k: np.asarray(v) for k, v in inputs.items()}
    nc = _get_nc()
    in_maps = make_in_maps(inputs)
    res = run_bass_kernel_spmd(nc, in_maps, list(range(N_CORES)))
    return _assemble(inputs, res.results)


if __name__ == "__main__":
    rng = np.random.default_rng(0)
    fake = dict(
        rgb_local=rng.standard_normal((B, C, H, W), dtype=np.float32),
        ir_local=rng.standard_normal((B, C, H, W), dtype=np.float32),
        rgb_dense=rng.standard_normal((B, C, H, W), dtype=np.float32),
        ir_dense=rng.standard_normal((B, C, H, W), dtype=np.float32),
        gate_rgb_w=rng.standard_normal((2, C * H * W), dtype=np.float32) * 1e-3,
        gate_rgb_b=rng.standard_normal(2).astype(np.float32),
        gate_ir_w=rng.standard_normal((2, C * H * W), dtype=np.float32) * 1e-3,
        gate_ir_b=rng.standard_normal(2).astype(np.float32),
        ew1=rng.standard_normal((E, CH, C, 3, 3), dtype=np.float32) * 0.05,
        eb1=rng.standard_normal((E, CH)).astype(np.float32),
        ew2=rng.standard_normal((E, C, CH, 3, 3), dtype=np.float32) * 0.05,
        eb2=rng.standard_normal((E, C)).astype(np.float32),
    )
    o = kernel(**fake)
    print("out shape:", o.shape, o.dtype)
